# revision 26
# baseline (speedup 1.0000x reference)
import math
import os

import numpy as np

import concourse.bass as bass
import concourse.mybir as mybir
import concourse.tile as tile
from concourse import bacc, bass_utils

B = 2
S = 2048
DIN = 2048
DOUT = 2048
LATENT = 512
HEADS = 16
HD = 128
NCORES = 8
GROUPS = 4
GH = HEADS // GROUPS
GD = GH * HD

SB = 256
NSB = S // SB
KT = DIN // 128
LT = LATENT // 128
QC = 512
NQC = S // QC
NKB = S // 128

F32 = mybir.dt.float32
F32R = mybir.dt.float32r
SCALE = 1.0 / math.sqrt(float(S))


def build_nc(stage=None, repeat=None):
    if stage is None:
        stage = int(os.environ.get("K_STAGE", "4"))
    if repeat is None:
        repeat = int(os.environ.get("K_REPEAT", "1"))
    nc = bacc.Bacc(
        "TRN2", target_bir_lowering=False, debug=False, num_devices=NCORES
    )
    _build_body(nc, stage, repeat)
    nc.compile()
    return nc


def _build_body(nc, stage, repeat=1):
    xT = nc.dram_tensor("xT", [DIN, S], F32R, kind="ExternalInput")
    wq = nc.dram_tensor("wq", [DIN, GD], F32R, kind="ExternalInput")
    wl = nc.dram_tensor("wl", [DIN, LATENT], F32R, kind="ExternalInput")
    wuk = nc.dram_tensor("wuk", [LATENT, GD], F32R, kind="ExternalInput")
    wuv = nc.dram_tensor("wuv", [LATENT, GD], F32R, kind="ExternalInput")
    wp = nc.dram_tensor("wp", [GD, DOUT], F32R, kind="ExternalInput")
    cosT = nc.dram_tensor("cosT", [HD, S], F32, kind="ExternalInput")
    sinT = nc.dram_tensor("sinT", [HD, S], F32, kind="ExternalInput")
    masksd = nc.dram_tensor("masks", [128, QC + 384], mybir.dt.bfloat16, kind="ExternalInput")
    onesd = nc.dram_tensor("ones", [128, 1], F32R, kind="ExternalInput")
    pout = nc.dram_tensor("pout", [S, DOUT], F32, kind="ExternalOutput")

    xT_t = xT.rearrange("(ko ki) s -> ki ko s", ki=128)
    wq_t = wq.rearrange("(ko ki) d -> ki ko d", ki=128)
    wl_t = wl.rearrange("(ko ki) l -> ki ko l", ki=128)
    wuk_t = wuk.rearrange("(lo li) d -> li lo d", li=128)
    wuv_t = wuv.rearrange("(lo li) d -> li lo d", li=128)
    wp_t = wp.rearrange("(dt di) e -> di dt e", di=128)

    with tile.TileContext(nc) as tc:
      for _rep in range(repeat):
        with (
            tc.tile_pool(name="persist", bufs=1) as persist,
            tc.tile_pool(name="kvres", bufs=1) as kvres,
        ):
            qres = tc.alloc_tile_pool(name="qres", bufs=1, side="right")
            xtp = tc.alloc_tile_pool(name="xt", bufs=2, side="right")
            cs = tc.alloc_tile_pool(name="cs", bufs=1, side="right")
            cos_sb = cs.tile([HD, S], F32)
            sin_sb = cs.tile([HD, S], F32)
            ones_sb = persist.tile([128, 1], F32R)
            nc.sync.dma_start(cos_sb[:], cosT[:, :])
            nc.sync.dma_start(sin_sb[:], sinT[:, :])
            nc.sync.dma_start(ones_sb[:], onesd[:, :])

            kT_c = [kvres.tile([128, GH, QC], F32R, tag=f"kT{c}", name=f"kT{c}")
                    for c in range(NQC)]
            v_c = [kvres.tile([128, 4, GD], F32R, tag=f"v{c}", name=f"v{c}")
                   for c in range(NQC)]

            def rope(dst, src_ps, tmp_pool, s0, n):
                tmp = tmp_pool.tile([128, SB], F32, tag="rope_tmp")
                nc.vector.tensor_mul(
                    tmp[0:64, :n], src_ps[64:128, :], sin_sb[0:64, s0:s0 + n]
                )
                nc.vector.tensor_mul(
                    tmp[64:128, :n], src_ps[0:64, :], sin_sb[64:128, s0:s0 + n]
                )
                nc.vector.tensor_mul(dst, src_ps[:, :], cos_sb[:, s0:s0 + n])
                nc.gpsimd.tensor_add(dst, dst, tmp[:, :n])

            with (
                tc.tile_pool(name="w1", bufs=1) as w1,
                tc.tile_pool(name="kvl", bufs=2) as kvlp,
                tc.tile_pool(name="tmp1", bufs=2) as tmp1,
                tc.tile_pool(name="ps1", bufs=2, space="PSUM") as ps1,
            ):
                wl_sb = w1.tile([128, KT, LATENT], F32R)
                wuk_sb = w1.tile([128, LT, GD], F32R)
                wuv_sb = w1.tile([128, LT, GD], F32R)
                xt0_sb = xtp.tile([128, KT, SB], F32R, tag="xt", name="xt0")
                for kg in range(4):
                    nc.sync.dma_start(
                        xt0_sb[:, 4 * kg:4 * kg + 4, :],
                        xT_t[:, 4 * kg:4 * kg + 4, 0:SB],
                    )
                for ko in range(KT):
                    nc.sync.dma_start(wl_sb[:, ko, :], wl_t[:, ko, :])
                nc.sync.dma_start(wuk_sb[:], wuk_t)
                nc.sync.dma_start(wuv_sb[:], wuv_t)

                for sb in range(NSB):
                    s0 = sb * SB
                    if sb == 0:
                        xt_sb = xt0_sb
                    else:
                        xt_sb = xtp.tile([128, KT, SB], F32R, tag="xt")
                        for kg in range(4):
                            nc.sync.dma_start(
                                xt_sb[:, 4 * kg:4 * kg + 4, :],
                                xT_t[:, 4 * kg:4 * kg + 4, s0:s0 + SB],
                            )

                    kvl_sb = kvlp.tile([128, LT, SB], F32R, tag="kvl")
                    for lo in range(LT):
                        ps = ps1.tile([128, SB], F32, tag="kvl_ps")
                        for ko in range(KT):
                            nc.tensor.matmul(
                                ps[:],
                                wl_sb[:, ko, lo * 128:(lo + 1) * 128],
                                xt_sb[:, ko, :],
                                start=(ko == 0),
                                stop=(ko == KT - 1),
                            )
                        nc.vector.tensor_copy(kvl_sb[:, lo, :], ps[:])

                    for hh in range(GH):
                        ps = ps1.tile([128, SB], F32, tag="kT_ps")
                        for lo in range(LT):
                            nc.tensor.matmul(
                                ps[:],
                                wuk_sb[:, lo, hh * 128:(hh + 1) * 128],
                                kvl_sb[:, lo, :],
                                start=(lo == 0),
                                stop=(lo == LT - 1),
                            )
                        rope(kT_c[s0 // QC][:, hh, s0 % QC:s0 % QC + SB],
                             ps, tmp1, s0, SB)

                    for sc in range(SB // 128):
                        j = (s0 + sc * 128) // 128
                        ps = ps1.tile([128, GD], F32, tag="v_ps")
                        for lo in range(LT):
                            nc.tensor.matmul(
                                ps[:],
                                kvl_sb[:, lo, sc * 128:(sc + 1) * 128],
                                wuv_sb[:, lo, :],
                                start=(lo == 0),
                                stop=(lo == LT - 1),
                            )
                        nc.vector.tensor_copy(v_c[j // 4][:, j % 4, :], ps[:])

            if stage <= 1:
                nc.sync.dma_start(pout[0:128, 0:512], v_c[0][:, 0, :].bitcast(F32))
                nc.sync.dma_start(
                    pout[128:256, 0:512], kT_c[0][:, 0, 0:512].bitcast(F32)
                )
                cs.release()
                xtp.release()
                qres.release()
                return

            with (
                tc.tile_pool(name="wqp", bufs=1) as wqp,
                tc.tile_pool(name="qtp", bufs=2) as qtp,
                tc.tile_pool(name="tmp2", bufs=2) as tmp2,
                tc.tile_pool(name="mk", bufs=1) as mkp,
                tc.tile_pool(name="att", bufs=3) as attp,
                tc.tile_pool(name="attr", bufs=2) as attrp,
                tc.tile_pool(name="ps2", bufs=2, space="PSUM") as ps2,
                tc.tile_pool(name="ps_lg", bufs=2, space="PSUM") as pslg,
                tc.tile_pool(name="ps_ot", bufs=2, space="PSUM") as psot,
                tc.tile_pool(name="ps_r", bufs=2, space="PSUM") as psr,
            ):
                wq_sb = wqp.tile([128, KT, GD], F32R)
                for ko in range(KT):
                    nc.sync.dma_start(wq_sb[:, ko, :], wq_t[:, ko, :])

                masks_sb = mkp.tile([128, QC + 384], mybir.dt.bfloat16)
                nc.sync.dma_start(masks_sb[:], masksd[:, :])

                oT_c = [qres.tile([128, GH, QC], F32R, tag=f"oT{c}",
                                  name=f"oT{c}") for c in range(NQC)]
                qT_c = [None] * NQC

                def q_chunk(c):
                    qT_c[c] = qtp.tile([128, GH, QC], F32R, tag="qTc",
                                       name=f"qT{c}")
                    for sb in (2 * c, 2 * c + 1):
                        s0 = sb * SB
                        xt_sb = xtp.tile([128, KT, SB], F32R, tag="xt",
                                         name="xt2")
                        for kg in range(4):
                            nc.sync.dma_start(
                                xt_sb[:, 4 * kg:4 * kg + 4, :],
                                xT_t[:, 4 * kg:4 * kg + 4, s0:s0 + SB],
                            )
                        for hh in range(GH):
                            ps = ps2.tile([128, SB], F32, tag="qT_ps")
                            for ko in range(KT):
                                nc.tensor.matmul(
                                    ps[:],
                                    wq_sb[:, ko, hh * 128:(hh + 1) * 128],
                                    xt_sb[:, ko, :],
                                    start=(ko == 0),
                                    stop=(ko == KT - 1),
                                )
                            rope(qT_c[c][:, hh, s0 % QC:s0 % QC + SB],
                                 ps, tmp2, s0, SB)

                def attn(qi):
                    q0 = qi * QC
                    njb = 4 * qi + 4
                    for hh in range(GH):
                        o_ps = psot.tile([128, QC], F32, tag="o_ps")
                        r_ps = psr.tile([1, QC], F32, tag="r_ps")
                        for j in range(njb):
                            t = j - 4 * qi
                            qoff = 0 if t < 1 else min(128 * t, QC - 256)
                            nw = QC - qoff
                            lg = pslg.tile([128, QC], F32, tag="lg")
                            nc.tensor.matmul(
                                lg[:, :nw],
                                kT_c[j // 4][:, hh,
                                             (j % 4) * 128:(j % 4 + 1) * 128],
                                qT_c[qi][:, hh, qoff:],
                                start=True,
                                stop=True,
                            )
                            e_sb = attp.tile([128, QC], F32R, tag="e")
                            nc.scalar.activation(
                                e_sb[:, :nw],
                                lg[:, :nw],
                                mybir.ActivationFunctionType.Exp,
                                scale=SCALE,
                            )
                            if t >= 0:
                                m0 = 384 - 128 * t + qoff
                                nc.vector.tensor_mul(
                                    e_sb[:, :nw], e_sb[:, :nw],
                                    masks_sb[:, m0:m0 + nw],
                                )
                            nc.tensor.matmul(
                                o_ps[:, qoff:],
                                v_c[j // 4][:, j % 4,
                                            hh * 128:(hh + 1) * 128],
                                e_sb[:, :nw],
                                start=(j == 0),
                                stop=(j == njb - 1),
                            )
                            nc.tensor.matmul(
                                r_ps[:, qoff:],
                                ones_sb[:],
                                e_sb[:, :nw],
                                start=(j == 0),
                                stop=(j == njb - 1),
                            )
                        r_sb = attrp.tile([1, QC], F32, tag="r_sb")
                        nc.vector.reciprocal(r_sb[:], r_ps[:])
                        rb_sb = attrp.tile([128, QC], F32, tag="rb", bufs=1)
                        nc.gpsimd.partition_broadcast(rb_sb[:], r_sb[:])
                        nc.vector.tensor_mul(
                            oT_c[qi][:, hh, :], o_ps[:], rb_sb[:]
                        )

                q_chunk(0)
                q_chunk(1)
                attn(0)
                q_chunk(2)
                attn(1)
                q_chunk(3)
                cs.release()
                xtp.release()
                attn(2)
                attn(3)

                if stage <= 2:
                    nc.sync.dma_start(
                        pout[256:384, 0:512], qT_c[0][:, 0, :].bitcast(F32)
                    )
                    return

            with (
                tc.tile_pool(name="w3", bufs=1) as w3,
                tc.tile_pool(name="osb", bufs=6) as osbp,
                tc.tile_pool(name="ps_po", bufs=6, space="PSUM") as pspo,
            ):
                wp_sb = w3.tile([128, LT, DOUT], F32R)
                for dt_ in range(LT):
                    nc.sync.dma_start(wp_sb[:, dt_, :], wp_t[:, dt_, :])
                for qi in range(NQC):
                    q0 = qi * QC
                    for sc in range(QC // 128):
                        for ec in range(DOUT // 512):
                            po = pspo.tile([128, 512], F32, tag="po")
                            for dt_ in range(LT):
                                nc.tensor.matmul(
                                    po[:],
                                    oT_c[qi][:, dt_, sc * 128:(sc + 1) * 128],
                                    wp_sb[:, dt_, ec * 512:(ec + 1) * 512],
                                    start=(dt_ == 0),
                                    stop=(dt_ == LT - 1),
                                )
                            out_sb = osbp.tile([128, 512], F32, tag="out")
                            nc.vector.tensor_copy(out_sb[:], po[:])
                            nc.sync.dma_start(
                                pout[q0 + sc * 128:q0 + (sc + 1) * 128,
                                     ec * 512:(ec + 1) * 512],
                                out_sb[:],
                            )
            qres.release()


_CACHE: dict = {}


def _get_nc():
    if "nc" not in _CACHE:
        _CACHE["nc"] = build_nc()
    return _CACHE["nc"]


def _host_inputs(x, position_embeddings, Wq, Wl, Wu, Wp):
    x = np.asarray(x, dtype=np.float32)
    pe = np.asarray(position_embeddings, dtype=np.float32)[:S]
    Wq = np.asarray(Wq, dtype=np.float32)
    Wl = np.asarray(Wl, dtype=np.float32)
    Wu = np.asarray(Wu, dtype=np.float32)
    Wp = np.asarray(Wp, dtype=np.float32)

    cos = np.ascontiguousarray(np.cos(pe).T)
    sinF = np.ascontiguousarray(np.sin(pe).T)
    sinF[: HD // 2] *= -1.0

    k = np.arange(128)[:, None]
    c = np.arange(QC + 384)[None, :]
    import ml_dtypes
    masks = np.ascontiguousarray((c - 384 >= k).astype(ml_dtypes.bfloat16))

    xTs = [np.ascontiguousarray(x[b].T) for b in range(B)]

    in_maps = []
    for c in range(NCORES):
        b, g = divmod(c, GROUPS)
        in_maps.append({
            "xT": xTs[b],
            "wq": np.ascontiguousarray(Wq[:, g * GD:(g + 1) * GD]),
            "wl": Wl,
            "wuk": np.ascontiguousarray(Wu[:, g * GD:(g + 1) * GD]),
            "wuv": np.ascontiguousarray(
                Wu[:, DOUT + g * GD:DOUT + (g + 1) * GD]
            ),
            "wp": np.ascontiguousarray(Wp[g * GD:(g + 1) * GD, :]),
            "cosT": cos,
            "sinT": sinF,
            "masks": masks,
            "ones": np.ones((128, 1), dtype=np.float32),
        })
    return in_maps


def run(x, position_embeddings, Wq, Wl, Wu, Wp, trace=False):
    nc = _get_nc()
    in_maps = _host_inputs(x, position_embeddings, Wq, Wl, Wu, Wp)
    res = bass_utils.run_bass_kernel_spmd(
        nc, in_maps, core_ids=list(range(NCORES)), trace=trace,
        trace_cores=list(range(NCORES)) if trace else None,
    )
    parts = [r["pout"] for r in res.results]
    out = np.empty((B, S, DOUT), dtype=np.float32)
    for b in range(B):
        out[b] = np.sum(
            np.stack(parts[b * GROUPS:(b + 1) * GROUPS]),
            axis=0, dtype=np.float64,
        ).astype(np.float32)
    return out, res


def kernel(x, position_embeddings, Wq, Wl, Wu, Wp):
    out, _ = run(x, position_embeddings, Wq, Wl, Wu, Wp, trace=False)
    return out
